# revision 1
# baseline (speedup 1.0000x reference)
"""LIF (leaky integrate-and-fire) spiking-neuron scan on 8 Trainium2 NeuronCores.

Reference semantics (per element, f32):
    h_t = v_{t-1} + (x_t - v_{t-1}) / 2        (tau = 2, v_reset = 0)
    s_t = (h_t >= 1)                           (spike, threshold v_th = 1)
    v_t = h_t * (1 - s_t)                      (hard reset)

Device formulation (verified bit-identical to the jax reference on the
graded input):  with w_t = v_{t-1} + x_t  (so h_t = w_t / 2):
    w  = v + x_t                      (DVE tensor_tensor add, f32)
    m  = (w < 2) * 0.5  -> bf16       (DVE tensor_scalar dual-op; {0, 0.5} exact)
    v' = w * m                        (DVE tensor_tensor mult, f32)
The bf16 m tile doubles as the stored output: m == 0  <=>  spike fired.
Host decodes spikes = (m_bits == 0).

Sharding: batch dim B=64 split across 8 cores (8 rows each); time stays
local (sequential scan).  DRAM layout is partition-major [128, T*512] so
every DMA segment is contiguous per partition (full HBM bandwidth).
"""

import os
import numpy as np

T, B, N = 64, 64, 8192
NCORES = 8
BL = B // NCORES          # batch rows per core
P = 128                   # SBUF partitions
F = (BL * N) // P         # free elems per partition per step  (512)

# timestep chunking: small first chunks prime the pipeline, then steady-state
LOAD_CHUNKS = [1, 1, 2, 4] + [8] * 7
assert sum(LOAD_CHUNKS) == T
ST = 4                    # store granularity (timesteps)
# GPSIMD element-stream split is a measured dead end: DVE tensor_tensor
# reads two tensors, so its second read port is the DVE/GpSimd shared port
# and every TT serializes against GPSIMD ops (245us vs 137us all-DVE).
FG = 0                    # free elems per step handled by GPSIMD (rest on DVE)
FD = F - FG

_built = {}


def _build():
    if "nc" in _built:
        return _built["nc"]

    from contextlib import ExitStack
    import concourse.mybir as mybir
    from concourse import bacc, tile

    # Slim the kernel-exit choreography: the stock exit is
    # drain -> all_engine_barrier -> clear sems -> all_engine_barrier; the
    # trailing barrier only orders the sem clears against later instructions,
    # of which there are none at kernel end (~3us saved).
    from concourse.vector_clock import ScopedClock

    def _slim_drain_and_barrier(self, tick_clock, wait_clock):
        drain_inst = self.nc.sync.drain()
        wait_clock.add_sem_waits(
            drain_inst.ins, ScopedClock({None: tick_clock.global_clock})
        )
        self.nc.all_engine_barrier()
        popped = self.nc._tile_sem_poison_stack.pop()
        assert popped is self._sem_poison
        self.nc.clear_and_free_semaphores(list(self.sems.allocated().values()))

    tile.TileContext._drain_and_barrier = _slim_drain_and_barrier

    nc = bacc.Bacc("TRN2", target_bir_lowering=False, debug=False)
    # partition-major layouts: [P, T*F] so per-partition bytes are contiguous
    x_ext = nc.dram_tensor("x", [P, T * F], mybir.dt.float32, kind="ExternalInput")
    m_ext = nc.dram_tensor("m", [P, T * F], mybir.dt.bfloat16, kind="ExternalOutput")

    add = mybir.AluOpType.add
    mult = mybir.AluOpType.mult
    is_lt = mybir.AluOpType.is_lt

    with tile.TileContext(nc) as tc:
        with ExitStack() as ctx:
            xp = ctx.enter_context(tc.tile_pool(name="xp", bufs=4))
            mp = ctx.enter_context(tc.tile_pool(name="mp", bufs=4))
            vp = ctx.enter_context(tc.tile_pool(name="vp", bufs=1))
            wp = ctx.enter_context(tc.tile_pool(name="wp", bufs=2))

            v = vp.tile([P, FD], mybir.dt.float32)
            nc.vector.memset(v[:], 0.0)
            if FG:
                vg = vp.tile([P, FG], mybir.dt.float32)
                nc.gpsimd.memset(vg[:], 0.0)

            # issue all loads up front (Tile pipelines them against compute,
            # bounded by xp bufs); chunk tiles are tagged identically so they
            # share the buffers at max-chunk size.  Loads alternate between
            # the two HWDGE rings (SP / ACT) so early chunks land faster.
            x_tiles = []
            t0 = 0
            for i, ch in enumerate(LOAD_CHUNKS):
                xt = xp.tile([P, ch * F], mybir.dt.float32, tag="xchunk")
                dma_eng = nc.sync if i % 2 == 0 else nc.scalar
                dma_eng.dma_start(out=xt[:], in_=x_ext[:, t0 * F:(t0 + ch) * F])
                x_tiles.append((t0, ch, xt))
                t0 += ch

            for (t0, ch, xt) in x_tiles:
                for k in range(ch):
                    t = t0 + k
                    if t % ST == 0:
                        mt = mp.tile([P, ST * F], mybir.dt.bfloat16, tag="mchunk")
                    xs = xt[:, k * F:k * F + FD]
                    ms = mt[:, (t % ST) * F:(t % ST) * F + FD]
                    if t == 0:
                        # v0 = 0 so w = x_0: feed x directly, skip the add
                        w_ap = xs
                    else:
                        w = wp.tile([P, FD], mybir.dt.float32)
                        nc.vector.tensor_tensor(w[:], v[:], xs, add)
                        w_ap = w[:]
                    nc.vector.tensor_scalar(ms, w_ap, 2.0, 0.5, is_lt, mult)
                    if t < T - 1:
                        # last step's state is never consumed
                        nc.vector.tensor_tensor(v[:], w_ap, ms, mult)
                    if FG:
                        # independent element stream on GPSIMD (no port
                        # contention with DVE 1x-mode tensor_tensor ops)
                        xsg = xt[:, k * F + FD:(k + 1) * F]
                        msg = mt[:, (t % ST) * F + FD:(t % ST + 1) * F]
                        if t == 0:
                            wg_ap = xsg
                        else:
                            wg = wp.tile([P, FG], mybir.dt.float32, tag="wg")
                            nc.gpsimd.tensor_tensor(wg[:], vg[:], xsg, add)
                            wg_ap = wg[:]
                        nc.gpsimd.tensor_scalar(msg, wg_ap, 2.0, 0.5, is_lt, mult)
                        if t < T - 1:
                            nc.gpsimd.tensor_tensor(vg[:], wg_ap, msg, mult)
                    if t == T - ST // 2 - 1:
                        # split the final store so the kernel-exit drain only
                        # waits on a half-size transfer
                        nc.scalar.dma_start(
                            out=m_ext[:, (t - ST // 2 + 1) * F:(t + 1) * F],
                            in_=mt[:, :(ST // 2) * F],
                        )
                    elif t == T - 1:
                        nc.sync.dma_start(
                            out=m_ext[:, (t - ST // 2 + 1) * F:(t + 1) * F],
                            in_=mt[:, (ST // 2) * F:],
                        )
                    elif t % ST == ST - 1:
                        dma_eng = nc.scalar if (t // ST) % 2 == 0 else nc.sync
                        dma_eng.dma_start(
                            out=m_ext[:, (t - ST + 1) * F:(t + 1) * F],
                            in_=mt[:],
                        )

    nc.compile()
    _built["nc"] = nc
    return nc


def _install_ntff_hook() -> bool:
    """Provide antenv.axon_hooks (absent in this image) so that
    run_bass_kernel_spmd(trace=True) can capture NTFF profiles via the
    ctypes hook that trn_agent_boot already implements."""
    try:
        from antenv.axon_hooks import get_axon_ntff_profile_hook  # noqa: F401
        return True
    except ImportError:
        pass
    try:
        import sys
        import types
        import antenv
        from trn_agent_boot.trn_boot import _ntff_profile_via_ctypes

        hook = _ntff_profile_via_ctypes("/opt/axon/libaxon_pjrt.so")
        if hook is None:
            return False
        mod = types.ModuleType("antenv.axon_hooks")
        state = {"hook": hook}
        mod.get_axon_ntff_profile_hook = lambda: state["hook"]
        mod.set_axon_ntff_profile_hook = lambda h: state.__setitem__("hook", h)
        sys.modules["antenv.axon_hooks"] = mod
        antenv.axon_hooks = mod
        return True
    except Exception:
        return False


def kernel(x: np.ndarray) -> np.ndarray:
    import concourse.bass_utils as bass_utils

    nc = _build()

    x = np.asarray(x)
    assert x.shape == (T, B, N) and x.dtype == np.float32

    in_maps = []
    for c in range(NCORES):
        # [T, BL*N] -> [T, P, F] -> [P, T, F] -> [P, T*F]  (partition-major)
        shard = (
            x[:, c * BL:(c + 1) * BL, :]
            .reshape(T, P, F)
            .transpose(1, 0, 2)
            .reshape(P, T * F)
        )
        in_maps.append({"x": np.ascontiguousarray(shard)})

    trace = bool(int(os.environ.get("LIF_TRACE", "0")))
    if trace:
        trace = _install_ntff_hook()
        # artifact upload has no bucket in this container; neuter it
        bass_utils.upload_artifacts = lambda tmpdir: tmpdir

    try:
        res = bass_utils.run_bass_kernel_spmd(
            nc, in_maps, list(range(NCORES)), trace=trace
        )
    except Exception:
        if not trace:
            raise
        res = bass_utils.run_bass_kernel_spmd(
            nc, in_maps, list(range(NCORES)), trace=False
        )
    _built["last_result"] = res

    out = np.empty((T, B, N), np.float32)
    for c in range(NCORES):
        m = np.asarray(res.results[c]["m"])          # bf16 [P, T*F]
        bits = m.view(np.uint16).reshape(P, T, F).transpose(1, 0, 2)
        spikes = (bits == 0).astype(np.float32).reshape(T, BL, N)
        out[:, c * BL:(c + 1) * BL, :] = spikes
    return out



# revision 2
# speedup vs baseline: 1.3644x; 1.3644x over previous
"""LIF (leaky integrate-and-fire) spiking-neuron scan on 8 Trainium2 NeuronCores.

Reference semantics (per element, f32):
    h_t = v_{t-1} + (x_t - v_{t-1}) / 2        (tau = 2, v_reset = 0)
    s_t = (h_t >= 1)                           (spike, threshold v_th = 1)
    v_t = h_t * (1 - s_t)                      (hard reset)

Device formulation: shifted pre-activation u_t = v_{t-1} + x_t - 2, so
s_t = (u_t >= 0) and the whole step is ONE fused custom-DVE op:
    u_t = (0.5 * u_{t-1} + 1) * (u_{t-1} < 0) + (x_t - 2)
(x - 2 is precomputed on host; u_0 = -2 encodes v_0 = 0.)  The custom op
(registered into concourse.dve_ops at import, compiled into the per-NEFF
DVE table) runs at 1 elem/cycle/lane fp32 — one ~691 ns instruction per
timestep, ~727 ns dependent-chain pitch, bit-exact f32 arithmetic.

Output: u_t cast to fp8e4 during the SWDGE store DMA.  Spike <=> u_t >= 0
<=> fp8 sign bit clear (sign survives rounding and saturation), so the
host decodes spikes = (u8_bits < 0x80).  21 MB HBM traffic per core
(16.8 in + 4.2 out) vs 46.5 us of DVE time: slightly DMA-bound.

Sharding: batch dim B=64 split across 8 cores (8 rows each); time stays
local.  DRAM layout is partition-major [128, T*512] so every DMA segment
is contiguous per partition.
"""

import os
import numpy as np

T, B, N = 64, 64, 8192
NCORES = 8
BL = B // NCORES          # batch rows per core
P = 128                   # SBUF partitions
F = (BL * N) // P         # free elems per partition per step  (512)

# timestep chunking: small first chunks prime the pipeline, then steady-state
LOAD_CHUNKS = [1, 1, 2, 4] + [8] * 7
assert sum(LOAD_CHUNKS) == T
ST = 8                    # store granularity (timesteps per SWDGE cast-store)

_built = {}


def _register_lif_op():
    from concourse import dve_ops
    from concourse.dve_spec import (
        Spec, Src0, Src1, C0, Zero, One, lower, _has_src1,
    )
    from concourse.dve_uop import DveOpSpec

    for op in dve_ops.OPS:
        if op.name == "LIF_STEP_ANT":
            return op

    body = (Src0 * C0 + One) * (Src0 < Zero) + Src1

    def ref(in0, in1, s0, s1, imm2):
        f = np.float32
        mask = (in0 < 0).astype(f)
        return ((in0 * f(s0) + f(1.0)) * mask + in1).astype(f)

    spec = Spec(body=body, reference=ref)
    name = "LIF_STEP_ANT"
    row = dve_ops._CUSTOM_DVE_ROW_BASE + len(dve_ops.OPS)
    shas = {}
    for ver in ("v3", "v4"):
        tmp = DveOpSpec(
            name=name, opcode=row, uops=lower(spec, ver=ver),
            rd1_en=_has_src1(spec),
        )
        shas[ver] = tmp.sha(ver)
    op = dve_ops.DveOp(name, spec, subdim=False, uops_sha=shas)
    dve_ops.OPS.append(op)
    dve_ops._SUB_OPCODE_FOR_NAME[name] = row
    dve_ops.CUSTOM_DVE_SPECS[name] = spec
    return op


def _build():
    if "nc" in _built:
        return _built["nc"]

    from contextlib import ExitStack
    import concourse.mybir as mybir
    from concourse import bacc, tile

    # Slim the kernel-exit choreography: the stock exit is
    # drain -> all_engine_barrier -> clear sems -> all_engine_barrier; the
    # trailing barrier only orders the sem clears against later instructions,
    # of which there are none at kernel end (~3us saved).
    from concourse.vector_clock import ScopedClock

    def _slim_drain_and_barrier(self, tick_clock, wait_clock):
        drain_inst = self.nc.sync.drain()
        wait_clock.add_sem_waits(
            drain_inst.ins, ScopedClock({None: tick_clock.global_clock})
        )
        self.nc.all_engine_barrier()
        popped = self.nc._tile_sem_poison_stack.pop()
        assert popped is self._sem_poison
        self.nc.clear_and_free_semaphores(list(self.sems.allocated().values()))

    tile.TileContext._drain_and_barrier = _slim_drain_and_barrier

    lif_op = _register_lif_op()

    nc = bacc.Bacc("TRN2", target_bir_lowering=False, debug=False)
    # partition-major layouts: [P, T*F] so per-partition bytes are contiguous
    x_ext = nc.dram_tensor("x", [P, T * F], mybir.dt.float32, kind="ExternalInput")
    u8_ext = nc.dram_tensor("u8", [P, T * F], mybir.dt.float8e4, kind="ExternalOutput")

    with tile.TileContext(nc) as tc:
        with ExitStack() as ctx:
            xp = ctx.enter_context(tc.tile_pool(name="xp", bufs=5))
            up = ctx.enter_context(tc.tile_pool(name="up", bufs=3))
            ip = ctx.enter_context(tc.tile_pool(name="ip", bufs=1))

            u0 = ip.tile([P, F], mybir.dt.float32)
            nc.vector.memset(u0[:], -2.0)

            # issue all loads up front (Tile pipelines them against compute,
            # bounded by xp bufs); chunk tiles are tagged identically so they
            # share the buffers at max-chunk size.  Loads alternate between
            # the two HWDGE rings (SP / ACT).
            x_tiles = []
            t0 = 0
            for i, ch in enumerate(LOAD_CHUNKS):
                xt = xp.tile([P, ch * F], mybir.dt.float32, tag="xchunk")
                dma_eng = nc.sync if i % 2 == 0 else nc.scalar
                dma_eng.dma_start(out=xt[:], in_=x_ext[:, t0 * F:(t0 + ch) * F])
                x_tiles.append((t0, ch, xt))
                t0 += ch

            prev = u0[:]
            uc = None
            for (t0, ch, xt) in x_tiles:
                for k in range(ch):
                    t = t0 + k
                    if t % ST == 0:
                        uc = up.tile([P, ST * F], mybir.dt.float32, tag="uchunk")
                    cur = uc[:, (t % ST) * F:(t % ST + 1) * F]
                    nc.vector._custom_dve(
                        lif_op, out=cur, in0=prev,
                        in1=xt[:, k * F:(k + 1) * F], s0=0.5,
                    )
                    prev = cur
                    if t % ST == ST - 1:
                        if t == T - 1:
                            # split the final store so the exit drain only
                            # waits on a half-size transfer
                            h = ST // 2
                            nc.gpsimd.dma_start(
                                out=u8_ext[:, (t - ST + 1) * F:(t - h + 1) * F],
                                in_=uc[:, :(ST - h) * F],
                            )
                            nc.gpsimd.dma_start(
                                out=u8_ext[:, (t - h + 1) * F:(t + 1) * F],
                                in_=uc[:, (ST - h) * F:],
                            )
                        else:
                            nc.gpsimd.dma_start(
                                out=u8_ext[:, (t - ST + 1) * F:(t + 1) * F],
                                in_=uc[:],
                            )

    nc.compile()
    _built["nc"] = nc
    return nc


def _install_ntff_hook() -> bool:
    """Provide antenv.axon_hooks (absent in this image) so that
    run_bass_kernel_spmd(trace=True) can capture NTFF profiles via the
    ctypes hook that trn_agent_boot already implements."""
    try:
        from antenv.axon_hooks import get_axon_ntff_profile_hook  # noqa: F401
        return True
    except ImportError:
        pass
    try:
        import sys
        import types
        import antenv
        from trn_agent_boot.trn_boot import _ntff_profile_via_ctypes

        hook = _ntff_profile_via_ctypes("/opt/axon/libaxon_pjrt.so")
        if hook is None:
            return False
        mod = types.ModuleType("antenv.axon_hooks")
        state = {"hook": hook}
        mod.get_axon_ntff_profile_hook = lambda: state["hook"]
        mod.set_axon_ntff_profile_hook = lambda h: state.__setitem__("hook", h)
        sys.modules["antenv.axon_hooks"] = mod
        antenv.axon_hooks = mod
        return True
    except Exception:
        return False


def kernel(x: np.ndarray) -> np.ndarray:
    import concourse.bass_utils as bass_utils

    nc = _build()

    x = np.asarray(x)
    assert x.shape == (T, B, N) and x.dtype == np.float32

    xs = x - np.float32(2.0)
    in_maps = []
    for c in range(NCORES):
        # [T, BL*N] -> [T, P, F] -> [P, T, F] -> [P, T*F]  (partition-major)
        shard = (
            xs[:, c * BL:(c + 1) * BL, :]
            .reshape(T, P, F)
            .transpose(1, 0, 2)
            .reshape(P, T * F)
        )
        in_maps.append({"x": np.ascontiguousarray(shard)})

    trace = bool(int(os.environ.get("LIF_TRACE", "0")))
    if trace:
        trace = _install_ntff_hook()
        # artifact upload has no bucket in this container; neuter it
        bass_utils.upload_artifacts = lambda tmpdir: tmpdir

    try:
        res = bass_utils.run_bass_kernel_spmd(
            nc, in_maps, list(range(NCORES)), trace=trace
        )
    except Exception:
        if not trace:
            raise
        res = bass_utils.run_bass_kernel_spmd(
            nc, in_maps, list(range(NCORES)), trace=False
        )
    _built["last_result"] = res

    out = np.empty((T, B, N), np.float32)
    for c in range(NCORES):
        u8 = np.asarray(res.results[c]["u8"])          # fp8e4 [P, T*F]
        bits = u8.view(np.uint8).reshape(P, T, F).transpose(1, 0, 2)
        spikes = (bits < 0x80).astype(np.float32).reshape(T, BL, N)
        out[:, c * BL:(c + 1) * BL, :] = spikes
    return out


# revision 5
# speedup vs baseline: 1.7280x; 1.2665x over previous
"""LIF (leaky integrate-and-fire) spiking-neuron scan on 8 Trainium2 NeuronCores.

Reference semantics (per element, f32):
    h_t = v_{t-1} + (x_t - v_{t-1}) / 2        (tau = 2, v_reset = 0)
    s_t = (h_t >= 1)                           (spike, threshold v_th = 1)
    v_t = h_t * (1 - s_t)                      (hard reset)

Device formulation: shifted pre-activation u_t = v_{t-1} + x_t - 2, so
s_t = (u_t >= 0) and the whole step is ONE fused custom-DVE op:
    u_t = (0.5 * u_{t-1} + 1) * (u_{t-1} < 0) + (x_t - 2)
(x - 2 is precomputed on host; u_0 = -2 encodes v_0 = 0.)  The custom op
(registered into concourse.dve_ops at import, compiled into the per-NEFF
DVE table) runs at 1 elem/cycle/lane fp32 — one ~691 ns instruction per
timestep, ~727 ns dependent-chain pitch, bit-exact f32 arithmetic.

Output: u_t cast fp32 -> fp8e4 by the (otherwise idle) ACT engine, then
stored via HWDGE.  Spike <=> u_t >= 0 <=> fp8 sign bit clear (sign
survives rounding and saturation), so the host decodes
spikes = (u8_bits < 0x80).  Casting on ACT instead of during the DMA
keeps the 16.8 MB of fp32 u-reads off the SDMA/SBUF-AXI budget: DMA
moves only 16.8 MB in + 4.2 MB out per core vs 46.5 us of DVE time.
Loads get a dedicated HWDGE ring (SP); casts + stores ride the ACT ring.

Sharding: batch dim B=64 split across 8 cores (8 rows each); time stays
local.  DRAM layout is partition-major [128, T*512] so every DMA segment
is contiguous per partition.
"""

import os
import numpy as np

T, B, N = 64, 64, 8192
NCORES = 8
BL = B // NCORES          # batch rows per core
P = 128                   # SBUF partitions
F = (BL * N) // P         # free elems per partition per step  (512)

# timestep chunking: small first chunks prime the pipeline, then steady-state
LOAD_CHUNKS = [1, 1, 2, 4] + [8] * 7
assert sum(LOAD_CHUNKS) == T
UC = 8                    # u-history chunk (timesteps per SBUF u buffer)
CAST = 2                  # timesteps per ACT fp32->fp8 cast op
ST = 4                    # timesteps per HWDGE fp8 store

_built = {}


def _register_lif_op():
    from concourse import dve_ops
    from concourse.dve_spec import (
        Spec, Src0, Src1, C0, Zero, One, lower, _has_src1,
    )
    from concourse.dve_uop import DveOpSpec

    for op in dve_ops.OPS:
        if op.name == "LIF_STEP_ANT":
            return op

    body = (Src0 * C0 + One) * (Src0 < Zero) + Src1

    def ref(in0, in1, s0, s1, imm2):
        f = np.float32
        mask = (in0 < 0).astype(f)
        return ((in0 * f(s0) + f(1.0)) * mask + in1).astype(f)

    spec = Spec(body=body, reference=ref)
    name = "LIF_STEP_ANT"
    row = dve_ops._CUSTOM_DVE_ROW_BASE + len(dve_ops.OPS)
    shas = {}
    for ver in ("v3", "v4"):
        tmp = DveOpSpec(
            name=name, opcode=row, uops=lower(spec, ver=ver),
            rd1_en=_has_src1(spec),
        )
        shas[ver] = tmp.sha(ver)
    op = dve_ops.DveOp(name, spec, subdim=False, uops_sha=shas)
    dve_ops.OPS.append(op)
    dve_ops._SUB_OPCODE_FOR_NAME[name] = row
    dve_ops.CUSTOM_DVE_SPECS[name] = spec
    return op


def _build():
    if "nc" in _built:
        return _built["nc"]

    from contextlib import ExitStack
    import concourse.mybir as mybir
    from concourse import bacc, tile

    # Slim the kernel-exit choreography: the stock exit is
    # drain -> all_engine_barrier -> clear sems -> all_engine_barrier; the
    # trailing barrier only orders the sem clears against later instructions,
    # of which there are none at kernel end (~3us saved).
    from concourse.vector_clock import ScopedClock

    def _slim_drain_and_barrier(self, tick_clock, wait_clock):
        drain_inst = self.nc.sync.drain()
        wait_clock.add_sem_waits(
            drain_inst.ins, ScopedClock({None: tick_clock.global_clock})
        )
        self.nc.all_engine_barrier()
        popped = self.nc._tile_sem_poison_stack.pop()
        assert popped is self._sem_poison
        self.nc.clear_and_free_semaphores(list(self.sems.allocated().values()))

    tile.TileContext._drain_and_barrier = _slim_drain_and_barrier

    lif_op = _register_lif_op()

    nc = bacc.Bacc("TRN2", target_bir_lowering=False, debug=False)
    # partition-major layouts: [P, T*F] so per-partition bytes are contiguous
    x_ext = nc.dram_tensor("x", [P, T * F], mybir.dt.float32, kind="ExternalInput")
    u8_ext = nc.dram_tensor("u8", [P, T * F], mybir.dt.float8e4, kind="ExternalOutput")

    Copy = mybir.ActivationFunctionType.Copy

    with tile.TileContext(nc) as tc:
        with ExitStack() as ctx:
            xp = ctx.enter_context(tc.tile_pool(name="xp", bufs=5))
            up = ctx.enter_context(tc.tile_pool(name="up", bufs=3))
            cp = ctx.enter_context(tc.tile_pool(name="cp", bufs=3))
            ip = ctx.enter_context(tc.tile_pool(name="ip", bufs=1))

            u0 = ip.tile([P, F], mybir.dt.float32)
            nc.vector.memset(u0[:], -2.0)

            # issue all loads up front on the SP HWDGE ring (Tile pipelines
            # them against compute, bounded by xp bufs); chunk tiles are
            # tagged identically so they share the buffers at max-chunk size.
            # The ACT ring is reserved for fp8 casts + output stores so load
            # issue is never blocked behind a multi-us ACTIVATE.
            x_tiles = []
            t0 = 0
            for i, ch in enumerate(LOAD_CHUNKS):
                xt = xp.tile([P, ch * F], mybir.dt.float32, tag="xchunk")
                nc.sync.dma_start(out=xt[:], in_=x_ext[:, t0 * F:(t0 + ch) * F])
                x_tiles.append((t0, ch, xt))
                t0 += ch

            prev = u0[:]
            uc = None
            c8 = None
            for (t0, ch, xt) in x_tiles:
                for k in range(ch):
                    t = t0 + k
                    if t % UC == 0:
                        uc = up.tile([P, UC * F], mybir.dt.float32, tag="uchunk")
                    cur = uc[:, (t % UC) * F:(t % UC + 1) * F]
                    nc.vector._custom_dve(
                        lif_op, out=cur, in0=prev,
                        in1=xt[:, k * F:(k + 1) * F], s0=0.5,
                    )
                    prev = cur
                    if t % ST == 0:
                        c8 = cp.tile([P, ST * F], mybir.dt.float8e4, tag="c8")
                    if t % CAST == CAST - 1:
                        # ACT cast fp32 -> fp8e4 (sign-exact; |u| << 448)
                        uq = t % UC
                        nc.scalar.activation(
                            c8[:, (t % ST - CAST + 1) * F:(t % ST + 1) * F],
                            uc[:, (uq - CAST + 1) * F:(uq + 1) * F],
                            Copy, bias=0.0, scale=1.0,
                        )
                    if t % ST == ST - 1:
                        nc.scalar.dma_start(
                            out=u8_ext[:, (t - ST + 1) * F:(t + 1) * F],
                            in_=c8[:],
                        )

    nc.compile()
    _built["nc"] = nc
    return nc


def _install_ntff_hook() -> bool:
    """Provide antenv.axon_hooks (absent in this image) so that
    run_bass_kernel_spmd(trace=True) can capture NTFF profiles via the
    ctypes hook that trn_agent_boot already implements."""
    try:
        from antenv.axon_hooks import get_axon_ntff_profile_hook  # noqa: F401
        return True
    except ImportError:
        pass
    try:
        import sys
        import types
        import antenv
        from trn_agent_boot.trn_boot import _ntff_profile_via_ctypes

        hook = _ntff_profile_via_ctypes("/opt/axon/libaxon_pjrt.so")
        if hook is None:
            return False
        mod = types.ModuleType("antenv.axon_hooks")
        state = {"hook": hook}
        mod.get_axon_ntff_profile_hook = lambda: state["hook"]
        mod.set_axon_ntff_profile_hook = lambda h: state.__setitem__("hook", h)
        sys.modules["antenv.axon_hooks"] = mod
        antenv.axon_hooks = mod
        return True
    except Exception:
        return False


def kernel(x: np.ndarray) -> np.ndarray:
    import concourse.bass_utils as bass_utils

    nc = _build()

    x = np.asarray(x)
    assert x.shape == (T, B, N) and x.dtype == np.float32

    xs = x - np.float32(2.0)
    in_maps = []
    for c in range(NCORES):
        # [T, BL*N] -> [T, P, F] -> [P, T, F] -> [P, T*F]  (partition-major)
        shard = (
            xs[:, c * BL:(c + 1) * BL, :]
            .reshape(T, P, F)
            .transpose(1, 0, 2)
            .reshape(P, T * F)
        )
        in_maps.append({"x": np.ascontiguousarray(shard)})

    trace = bool(int(os.environ.get("LIF_TRACE", "0")))
    if trace:
        trace = _install_ntff_hook()
        # artifact upload has no bucket in this container; neuter it
        bass_utils.upload_artifacts = lambda tmpdir: tmpdir

    try:
        res = bass_utils.run_bass_kernel_spmd(
            nc, in_maps, list(range(NCORES)), trace=trace
        )
    except Exception:
        if not trace:
            raise
        res = bass_utils.run_bass_kernel_spmd(
            nc, in_maps, list(range(NCORES)), trace=False
        )
    _built["last_result"] = res

    out = np.empty((T, B, N), np.float32)
    for c in range(NCORES):
        u8 = np.asarray(res.results[c]["u8"])          # fp8e4 [P, T*F]
        bits = u8.view(np.uint8).reshape(P, T, F).transpose(1, 0, 2)
        spikes = (bits < 0x80).astype(np.float32).reshape(T, BL, N)
        out[:, c * BL:(c + 1) * BL, :] = spikes
    return out


# revision 7
# speedup vs baseline: 1.7724x; 1.0257x over previous
"""LIF (leaky integrate-and-fire) spiking-neuron scan on 8 Trainium2 NeuronCores.

Reference semantics (per element, f32):
    h_t = v_{t-1} + (x_t - v_{t-1}) / 2        (tau = 2, v_reset = 0)
    s_t = (h_t >= 1)                           (spike, threshold v_th = 1)
    v_t = h_t * (1 - s_t)                      (hard reset)

Device formulation: shifted pre-activation u_t = v_{t-1} + x_t - 2, so
s_t = (u_t >= 0) and the whole step is ONE fused custom-DVE op:
    u_t = (0.5 * u_{t-1} + 1) * (u_{t-1} < 0) + (x_t - 2)
(x - 2 is precomputed on host; u_0 = -2 encodes v_0 = 0.)  The custom op
(registered into concourse.dve_ops at import, compiled into the per-NEFF
DVE table) runs at 1 elem/cycle/lane fp32 — one ~691 ns instruction per
timestep, ~727 ns dependent-chain pitch, bit-exact f32 arithmetic.

Output: u_t cast fp32 -> fp8e4 by the (otherwise idle) ACT engine, then
stored via HWDGE.  Spike <=> u_t >= 0 <=> fp8 sign bit clear (sign
survives rounding and saturation), so the host decodes
spikes = (u8_bits < 0x80).  Casting on ACT instead of during the DMA
keeps the 16.8 MB of fp32 u-reads off the SDMA/SBUF-AXI budget: DMA
moves only 16.8 MB in + 4.2 MB out per core vs 46.5 us of DVE time.
Loads get a dedicated HWDGE ring (SP); casts + stores ride the ACT ring.

Sharding: batch dim B=64 split across 8 cores (8 rows each); time stays
local.  DRAM layout is partition-major [128, T*512] so every DMA segment
is contiguous per partition.
"""

import os
import numpy as np

T, B, N = 64, 64, 8192
NCORES = 8
BL = B // NCORES          # batch rows per core
P = 128                   # SBUF partitions
F = (BL * N) // P         # free elems per partition per step  (512)

# timestep chunking: small first chunks prime the pipeline, then steady-state
LOAD_CHUNKS = [1, 1, 2, 4] + [8] * 7
assert sum(LOAD_CHUNKS) == T
UC = 8                    # u-history chunk (timesteps per SBUF u buffer)
CAST = 4                  # timesteps per ACT fp32->fp8 cast op
ST = 8                    # timesteps per HWDGE fp8 store
TAPER = 2                 # cast/store granularity for the final chunk

_built = {}


def _register_lif_op():
    from concourse import dve_ops
    from concourse.dve_spec import (
        Spec, Src0, Src1, C0, Zero, One, lower, _has_src1,
    )
    from concourse.dve_uop import DveOpSpec

    for op in dve_ops.OPS:
        if op.name == "LIF_STEP_ANT":
            return op

    body = (Src0 * C0 + One) * (Src0 < Zero) + Src1

    def ref(in0, in1, s0, s1, imm2):
        f = np.float32
        mask = (in0 < 0).astype(f)
        return ((in0 * f(s0) + f(1.0)) * mask + in1).astype(f)

    spec = Spec(body=body, reference=ref)
    name = "LIF_STEP_ANT"
    row = dve_ops._CUSTOM_DVE_ROW_BASE + len(dve_ops.OPS)
    shas = {}
    for ver in ("v3", "v4"):
        tmp = DveOpSpec(
            name=name, opcode=row, uops=lower(spec, ver=ver),
            rd1_en=_has_src1(spec),
        )
        shas[ver] = tmp.sha(ver)
    op = dve_ops.DveOp(name, spec, subdim=False, uops_sha=shas)
    dve_ops.OPS.append(op)
    dve_ops._SUB_OPCODE_FOR_NAME[name] = row
    dve_ops.CUSTOM_DVE_SPECS[name] = spec
    return op


def _build():
    if "nc" in _built:
        return _built["nc"]

    from contextlib import ExitStack
    import concourse.mybir as mybir
    from concourse import bacc, tile

    # Slim the kernel-exit choreography: the stock exit is
    # drain -> all_engine_barrier -> clear sems -> all_engine_barrier; the
    # trailing barrier only orders the sem clears against later instructions,
    # of which there are none at kernel end (~3us saved).
    from concourse.vector_clock import ScopedClock

    def _slim_drain_and_barrier(self, tick_clock, wait_clock):
        drain_inst = self.nc.sync.drain()
        wait_clock.add_sem_waits(
            drain_inst.ins, ScopedClock({None: tick_clock.global_clock})
        )
        self.nc.all_engine_barrier()
        popped = self.nc._tile_sem_poison_stack.pop()
        assert popped is self._sem_poison
        self.nc.clear_and_free_semaphores(list(self.sems.allocated().values()))

    tile.TileContext._drain_and_barrier = _slim_drain_and_barrier

    lif_op = _register_lif_op()

    nc = bacc.Bacc("TRN2", target_bir_lowering=False, debug=False)
    # partition-major layouts: [P, T*F] so per-partition bytes are contiguous
    x_ext = nc.dram_tensor("x", [P, T * F], mybir.dt.float32, kind="ExternalInput")
    u8_ext = nc.dram_tensor("u8", [P, T * F], mybir.dt.float8e4, kind="ExternalOutput")

    Copy = mybir.ActivationFunctionType.Copy

    with tile.TileContext(nc) as tc:
        with ExitStack() as ctx:
            xp = ctx.enter_context(tc.tile_pool(name="xp", bufs=5))
            up = ctx.enter_context(tc.tile_pool(name="up", bufs=3))
            cp = ctx.enter_context(tc.tile_pool(name="cp", bufs=3))
            ip = ctx.enter_context(tc.tile_pool(name="ip", bufs=1))

            u0 = ip.tile([P, F], mybir.dt.float32)
            nc.vector.memset(u0[:], -2.0)

            # issue all loads up front, alternating the two HWDGE rings
            # (Tile pipelines them against compute, bounded by xp bufs);
            # chunk tiles are tagged identically so they share the buffers
            # at max-chunk size.
            x_tiles = []
            t0 = 0
            for i, ch in enumerate(LOAD_CHUNKS):
                xt = xp.tile([P, ch * F], mybir.dt.float32, tag="xchunk")
                dma_eng = nc.sync if i % 2 == 0 else nc.scalar
                dma_eng.dma_start(out=xt[:], in_=x_ext[:, t0 * F:(t0 + ch) * F])
                x_tiles.append((t0, ch, xt))
                t0 += ch

            prev = u0[:]
            uc = None
            c8 = None
            nstore = 0
            for (t0, ch, xt) in x_tiles:
                for k in range(ch):
                    t = t0 + k
                    # the final chunk tapers to small cast/store pieces so
                    # the post-compute tail is tiny
                    cast = CAST if t < T - UC else TAPER
                    st = ST if t < T - UC else TAPER
                    if t % UC == 0:
                        uc = up.tile([P, UC * F], mybir.dt.float32, tag="uchunk")
                    cur = uc[:, (t % UC) * F:(t % UC + 1) * F]
                    nc.vector._custom_dve(
                        lif_op, out=cur, in0=prev,
                        in1=xt[:, k * F:(k + 1) * F], s0=0.5,
                    )
                    prev = cur
                    if t % st == 0:
                        c8 = cp.tile([P, st * F], mybir.dt.float8e4, tag="c8")
                    if t % cast == cast - 1:
                        # ACT cast fp32 -> fp8e4 (sign-exact; |u| << 448)
                        uq = t % UC
                        nc.scalar.activation(
                            c8[:, (t % st - cast + 1) * F:(t % st + 1) * F],
                            uc[:, (uq - cast + 1) * F:(uq + 1) * F],
                            Copy, bias=0.0, scale=1.0,
                        )
                    if t % st == st - 1:
                        dma_eng = nc.scalar if nstore % 2 == 0 else nc.sync
                        nstore += 1
                        dma_eng.dma_start(
                            out=u8_ext[:, (t - st + 1) * F:(t + 1) * F],
                            in_=c8[:],
                        )

    nc.compile()
    _built["nc"] = nc
    return nc


def _install_ntff_hook() -> bool:
    """Provide antenv.axon_hooks (absent in this image) so that
    run_bass_kernel_spmd(trace=True) can capture NTFF profiles via the
    ctypes hook that trn_agent_boot already implements."""
    try:
        from antenv.axon_hooks import get_axon_ntff_profile_hook  # noqa: F401
        return True
    except ImportError:
        pass
    try:
        import sys
        import types
        import antenv
        from trn_agent_boot.trn_boot import _ntff_profile_via_ctypes

        hook = _ntff_profile_via_ctypes("/opt/axon/libaxon_pjrt.so")
        if hook is None:
            return False
        mod = types.ModuleType("antenv.axon_hooks")
        state = {"hook": hook}
        mod.get_axon_ntff_profile_hook = lambda: state["hook"]
        mod.set_axon_ntff_profile_hook = lambda h: state.__setitem__("hook", h)
        sys.modules["antenv.axon_hooks"] = mod
        antenv.axon_hooks = mod
        return True
    except Exception:
        return False


def kernel(x: np.ndarray) -> np.ndarray:
    import concourse.bass_utils as bass_utils

    nc = _build()

    x = np.asarray(x)
    assert x.shape == (T, B, N) and x.dtype == np.float32

    xs = x - np.float32(2.0)
    in_maps = []
    for c in range(NCORES):
        # [T, BL*N] -> [T, P, F] -> [P, T, F] -> [P, T*F]  (partition-major)
        shard = (
            xs[:, c * BL:(c + 1) * BL, :]
            .reshape(T, P, F)
            .transpose(1, 0, 2)
            .reshape(P, T * F)
        )
        in_maps.append({"x": np.ascontiguousarray(shard)})

    trace = bool(int(os.environ.get("LIF_TRACE", "0")))
    if trace:
        trace = _install_ntff_hook()
        # artifact upload has no bucket in this container; neuter it
        bass_utils.upload_artifacts = lambda tmpdir: tmpdir

    try:
        res = bass_utils.run_bass_kernel_spmd(
            nc, in_maps, list(range(NCORES)), trace=trace
        )
    except Exception:
        if not trace:
            raise
        res = bass_utils.run_bass_kernel_spmd(
            nc, in_maps, list(range(NCORES)), trace=False
        )
    _built["last_result"] = res

    out = np.empty((T, B, N), np.float32)
    for c in range(NCORES):
        u8 = np.asarray(res.results[c]["u8"])          # fp8e4 [P, T*F]
        bits = u8.view(np.uint8).reshape(P, T, F).transpose(1, 0, 2)
        spikes = (bits < 0x80).astype(np.float32).reshape(T, BL, N)
        out[:, c * BL:(c + 1) * BL, :] = spikes
    return out


# revision 9
# speedup vs baseline: 1.8564x; 1.0474x over previous
"""LIF (leaky integrate-and-fire) spiking-neuron scan on 8 Trainium2 NeuronCores.

Reference semantics (per element, f32):
    h_t = v_{t-1} + (x_t - v_{t-1}) / 2        (tau = 2, v_reset = 0)
    s_t = (h_t >= 1)                           (spike, threshold v_th = 1)
    v_t = h_t * (1 - s_t)                      (hard reset)

Device formulation: shifted pre-activation u_t = v_{t-1} + x_t - 2, so
s_t = (u_t >= 0) and the whole step is ONE fused custom-DVE op:
    u_t = (0.5 * u_{t-1} + 1) * (u_{t-1} < 0) + (x_t - 2)
(x - 2 is precomputed on host; u_0 = -2 encodes v_0 = 0.)  The custom op
(registered into concourse.dve_ops at import, compiled into the per-NEFF
DVE table) runs at 1 elem/cycle/lane fp32 — one ~691 ns instruction per
timestep, ~727 ns dependent-chain pitch, bit-exact f32 arithmetic.

Output: u_t cast fp32 -> fp8e4 by the (otherwise idle) ACT engine, then
stored via HWDGE.  Spike <=> u_t >= 0 <=> fp8 sign bit clear (sign
survives rounding and saturation), so the host decodes
spikes = (u8_bits < 0x80).  Casting on ACT instead of during the DMA
keeps the 16.8 MB of fp32 u-reads off the SDMA/SBUF-AXI budget: DMA
moves only 16.8 MB in + 4.2 MB out per core vs 46.5 us of DVE time.
Loads get a dedicated HWDGE ring (SP); casts + stores ride the ACT ring.

Sharding: batch dim B=64 split across 8 cores (8 rows each); time stays
local.  DRAM layout is partition-major [128, T*512] so every DMA segment
is contiguous per partition.
"""

import os
import numpy as np

T, B, N = 64, 64, 8192
NCORES = 8
BL = B // NCORES          # batch rows per core
P = 128                   # SBUF partitions
F = (BL * N) // P         # free elems per partition per step  (512)

# timestep chunking: small first chunks prime the pipeline, then steady-state
LOAD_CHUNKS = [1, 1, 2, 4] + [4] * 14
assert sum(LOAD_CHUNKS) == T
UC = 8                    # u-history chunk (timesteps per SBUF u buffer)
CAST = 4                  # timesteps per ACT fp32->fp8 cast op
TAPER = 1                 # cast granularity for the last TAIL steps
TAIL = 4                  # final steps cast one-by-one to shrink the tail

_built = {}


def _register_lif_op():
    from concourse import dve_ops
    from concourse.dve_spec import (
        Spec, Src0, Src1, C0, Zero, One, lower, _has_src1,
    )
    from concourse.dve_uop import DveOpSpec

    for op in dve_ops.OPS:
        if op.name == "LIF_STEP_ANT":
            return op

    body = (Src0 * C0 + One) * (Src0 < Zero) + Src1

    def ref(in0, in1, s0, s1, imm2):
        f = np.float32
        mask = (in0 < 0).astype(f)
        return ((in0 * f(s0) + f(1.0)) * mask + in1).astype(f)

    spec = Spec(body=body, reference=ref)
    name = "LIF_STEP_ANT"
    row = dve_ops._CUSTOM_DVE_ROW_BASE + len(dve_ops.OPS)
    shas = {}
    for ver in ("v3", "v4"):
        tmp = DveOpSpec(
            name=name, opcode=row, uops=lower(spec, ver=ver),
            rd1_en=_has_src1(spec),
        )
        shas[ver] = tmp.sha(ver)
    op = dve_ops.DveOp(name, spec, subdim=False, uops_sha=shas)
    dve_ops.OPS.append(op)
    dve_ops._SUB_OPCODE_FOR_NAME[name] = row
    dve_ops.CUSTOM_DVE_SPECS[name] = spec
    return op


def _build():
    if "nc" in _built:
        return _built["nc"]

    from contextlib import ExitStack
    import concourse.mybir as mybir
    from concourse import bacc, tile

    # Slim the kernel-exit choreography: the stock exit is
    # drain -> all_engine_barrier -> clear sems -> all_engine_barrier; the
    # trailing barrier only orders the sem clears against later instructions,
    # of which there are none at kernel end (~3us saved).
    from concourse.vector_clock import ScopedClock

    def _slim_drain_and_barrier(self, tick_clock, wait_clock):
        drain_inst = self.nc.sync.drain()
        wait_clock.add_sem_waits(
            drain_inst.ins, ScopedClock({None: tick_clock.global_clock})
        )
        self.nc.all_engine_barrier()
        popped = self.nc._tile_sem_poison_stack.pop()
        assert popped is self._sem_poison
        self.nc.clear_and_free_semaphores(list(self.sems.allocated().values()))

    tile.TileContext._drain_and_barrier = _slim_drain_and_barrier

    lif_op = _register_lif_op()

    nc = bacc.Bacc("TRN2", target_bir_lowering=False, debug=False)
    # partition-major layouts: [P, T*F] so per-partition bytes are contiguous
    x_ext = nc.dram_tensor("x", [P, T * F], mybir.dt.float32, kind="ExternalInput")
    u8_ext = nc.dram_tensor("u8", [P, T * F], mybir.dt.float8e4, kind="ExternalOutput")

    Copy = mybir.ActivationFunctionType.Copy

    # scratch target for the store-gate dummy DMA
    gate_ext = nc.dram_tensor("gate", [1, 1], mybir.dt.float32, kind="Internal")

    with tile.TileContext(nc) as tc:
        with ExitStack() as ctx:
            xp = ctx.enter_context(tc.tile_pool(name="xp", bufs=7))
            up = ctx.enter_context(tc.tile_pool(name="up", bufs=3))
            cp = ctx.enter_context(tc.tile_pool(name="cp", bufs=9))
            ip = ctx.enter_context(tc.tile_pool(name="ip", bufs=1))

            u0 = ip.tile([P, F], mybir.dt.float32)
            nc.vector.memset(u0[:], -2.0)

            # issue all loads up front, alternating the two HWDGE rings
            # (Tile pipelines them against compute, bounded by xp bufs);
            # chunk tiles are tagged identically so they share the buffers
            # at max-chunk size.
            x_tiles = []
            t0 = 0
            for i, ch in enumerate(LOAD_CHUNKS):
                xt = xp.tile([P, ch * F], mybir.dt.float32, tag="xchunk")
                dma_eng = nc.sync if i % 2 == 0 else nc.scalar
                dma_eng.dma_start(out=xt[:], in_=x_ext[:, t0 * F:(t0 + ch) * F])
                x_tiles.append((t0, ch, xt))
                t0 += ch

            prev = u0[:]
            uc = None
            c8_tiles = []   # (t_first, nsteps, tile) pending output stores
            c8 = None
            c8_t0 = 0
            c8_n = 0
            for (t0, ch, xt) in x_tiles:
                for k in range(ch):
                    t = t0 + k
                    # final steps cast one-by-one so the post-compute tail
                    # is a single small cast + store
                    cast = CAST if t < T - TAIL else TAPER
                    if t % UC == 0:
                        uc = up.tile([P, UC * F], mybir.dt.float32, tag="uchunk")
                    cur = uc[:, (t % UC) * F:(t % UC + 1) * F]
                    nc.vector._custom_dve(
                        lif_op, out=cur, in0=prev,
                        in1=xt[:, k * F:(k + 1) * F], s0=0.5,
                    )
                    prev = cur
                    if t % UC == 0:
                        c8 = cp.tile([P, UC * F], mybir.dt.float8e4, tag="c8")
                        c8_t0 = t
                        c8_n = 0
                    if (t + 1) % cast == 0:
                        # ACT cast fp32 -> fp8e4 (sign-exact; |u| << 448)
                        uq = t % UC
                        nc.scalar.activation(
                            c8[:, (uq - cast + 1) * F:(uq + 1) * F],
                            uc[:, (uq - cast + 1) * F:(uq + 1) * F],
                            Copy, bias=0.0, scale=1.0,
                        )
                        c8_n += cast
                    if t % UC == UC - 1:
                        c8_tiles.append((c8_t0, c8_n, c8))

            # store gate: a dummy DMA whose semaphore wait (on the final x
            # chunk's load) blocks the sync queue, deferring every output
            # store until ALL input loads have finished.  Keeps the fp8
            # stores from stealing HBM/SBUF-AXI bandwidth during the load
            # phase; they burst at full rate under the last compute steps.
            last_xt = x_tiles[-1][2]
            nc.sync.dma_start(out=gate_ext[:, :], in_=last_xt[:1, :1])
            for (ct0, cn, ct) in c8_tiles:
                # split each chunk store in two for finer completion overlap
                h = cn // 2
                nc.sync.dma_start(
                    out=u8_ext[:, ct0 * F:(ct0 + h) * F], in_=ct[:, :h * F]
                )
                nc.sync.dma_start(
                    out=u8_ext[:, (ct0 + h) * F:(ct0 + cn) * F],
                    in_=ct[:, h * F:cn * F],
                )

    nc.compile()
    _built["nc"] = nc
    return nc


def _install_ntff_hook() -> bool:
    """Provide antenv.axon_hooks (absent in this image) so that
    run_bass_kernel_spmd(trace=True) can capture NTFF profiles via the
    ctypes hook that trn_agent_boot already implements."""
    try:
        from antenv.axon_hooks import get_axon_ntff_profile_hook  # noqa: F401
        return True
    except ImportError:
        pass
    try:
        import sys
        import types
        import antenv
        from trn_agent_boot.trn_boot import _ntff_profile_via_ctypes

        hook = _ntff_profile_via_ctypes("/opt/axon/libaxon_pjrt.so")
        if hook is None:
            return False
        mod = types.ModuleType("antenv.axon_hooks")
        state = {"hook": hook}
        mod.get_axon_ntff_profile_hook = lambda: state["hook"]
        mod.set_axon_ntff_profile_hook = lambda h: state.__setitem__("hook", h)
        sys.modules["antenv.axon_hooks"] = mod
        antenv.axon_hooks = mod
        return True
    except Exception:
        return False


def kernel(x: np.ndarray) -> np.ndarray:
    import concourse.bass_utils as bass_utils

    nc = _build()

    x = np.asarray(x)
    assert x.shape == (T, B, N) and x.dtype == np.float32

    xs = x - np.float32(2.0)
    in_maps = []
    for c in range(NCORES):
        # [T, BL*N] -> [T, P, F] -> [P, T, F] -> [P, T*F]  (partition-major)
        shard = (
            xs[:, c * BL:(c + 1) * BL, :]
            .reshape(T, P, F)
            .transpose(1, 0, 2)
            .reshape(P, T * F)
        )
        in_maps.append({"x": np.ascontiguousarray(shard)})

    trace = bool(int(os.environ.get("LIF_TRACE", "0")))
    if trace:
        trace = _install_ntff_hook()
        # artifact upload has no bucket in this container; neuter it
        bass_utils.upload_artifacts = lambda tmpdir: tmpdir

    try:
        res = bass_utils.run_bass_kernel_spmd(
            nc, in_maps, list(range(NCORES)), trace=trace
        )
    except Exception:
        if not trace:
            raise
        res = bass_utils.run_bass_kernel_spmd(
            nc, in_maps, list(range(NCORES)), trace=False
        )
    _built["last_result"] = res

    out = np.empty((T, B, N), np.float32)
    for c in range(NCORES):
        u8 = np.asarray(res.results[c]["u8"])          # fp8e4 [P, T*F]
        bits = u8.view(np.uint8).reshape(P, T, F).transpose(1, 0, 2)
        spikes = (bits < 0x80).astype(np.float32).reshape(T, BL, N)
        out[:, c * BL:(c + 1) * BL, :] = spikes
    return out
